# revision 40
# baseline (speedup 1.0000x reference)
"""Causal self-attention (GQA + RoPE) Trainium2 Bass kernel, 8-way sharded.

Sharding: core c -> batch b = c // 2, head-half hh = c % 2.
Each core computes qkv projection, attention and output projection for its
batch and its 16 query heads / 4 kv heads; the output projection is a
row-shard of Wproj, so the two cores of a batch produce partial sums that
the host adds.

Design (all-SBUF, bf16; TimelineSim ~636us/core, PE ~97% busy at ~97% of
the 1-col/cycle matmul roofline):
  - q/k projection runs W-stationary (moving operand = xT), so the PE emits
    q^T/k^T [hd, T] directly -- no PE transposes, no DRAM round trip. RoPE
    (rotate-half form via host-side column de-interleave) is applied on the
    PSUM->SBUF path by ACT (half swap) + DVE (ps*cos) + Pool (sw*sin) +
    DVE (add, bf16 out).
  - v runs x-stationary into natural [T, 4*HD] layout; its 8 psum chains
    ride inside the group-0 slots (3 concurrently during the DMA-paced
    startup window, in per-x-chunk lockstep with the k0 chain).
  - attention uses transposed scores (scoresT = kT_slice^T @ qT) so no
    transposes are needed anywhere. Softmax: exp on ACT (PSUM->SBUF, bf16),
    si-pair exp tiles tree-summed on DVE, then ONE ones-matrix matmul per
    256-col chunk yields the denominator already broadcast across
    partitions (replaces per-si reduction matmuls + broadcast matmuls).
    The odd-diagonal 128-block is trimmed to half-width matmuls/exp.
  - attention for kv-group g is emitted interleaved (fine-grained, between
    qkv cc-steps -- engine queues are FIFO, so emission order IS schedule)
    into the qkv slots of group g+1; group 3 additionally schedules into
    its own late slots, and its tail overlaps the first three projection
    accumulations (ycc 0-11, which don't depend on the last heads).
  - projection reads SBUF-resident bf16 y^T tiles; everything bf16 except
    PSUM accumulation (fp32) and the final output.
"""

import os

os.environ.setdefault("JAX_PLATFORMS", "axon")

import numpy as np
import ml_dtypes

BF16 = ml_dtypes.bfloat16

B, T, C = 4, 1024, 4096
H, KV, HD = 32, 8, 128
REP = H // KV  # 4

NQ = 16      # q heads per core
NKV = 4      # kv heads per core
QK_HEADS = NQ + NKV   # 20 projected+rope'd heads per core
NCC = C // 128        # 32 contraction tiles
NTT = T // 128        # 8 token tiles
SCALE = float(1.0 / np.sqrt(np.float32(HD)).astype(np.float32))

_CACHE: dict = {}


def _build_nc():
    import concourse.mybir as mybir
    import concourse.tile as tile
    from concourse import bacc
    from concourse.bass import ts

    f32 = mybir.dt.float32
    bf16 = mybir.dt.bfloat16
    Exp = mybir.ActivationFunctionType.Exp

    nc = bacc.Bacc(None, target_bir_lowering=False, debug=False)

    xT_d = nc.dram_tensor("xT", [C, T], bf16, kind="ExternalInput")
    # [h, p(c within tile), cc, col]
    wqk_d = nc.dram_tensor("wqk", [QK_HEADS, 128, NCC, 128], bf16,
                           kind="ExternalInput")
    # [p, cc, vcol]
    wv_d = nc.dram_tensor("wv", [128, NCC, NKV * HD], bf16, kind="ExternalInput")
    # [ccol, p(y within tile), ycc, f]
    wp_d = nc.dram_tensor("wp", [8, 128, 16, 512], bf16, kind="ExternalInput")
    csg_d = nc.dram_tensor("csg", [128, T], bf16, kind="ExternalInput")
    ssg_d = nc.dram_tensor("ssg", [128, T], bf16, kind="ExternalInput")
    mask_d = nc.dram_tensor("maskd", [128, 2, 256], bf16, kind="ExternalInput")
    out_d = nc.dram_tensor("out", [T, C], f32, kind="ExternalOutput")

    with (
        tile.TileContext(nc) as tc,
        tc.tile_pool(name="const", bufs=1) as const_p,
        tc.tile_pool(name="qkT", bufs=10) as qkT_p,
        tc.tile_pool(name="yts", bufs=NQ) as yts_p,
        tc.tile_pool(name="vsb", bufs=1) as vsb_p,
        tc.tile_pool(name="sw", bufs=2) as sw_p,
        tc.tile_pool(name="m1", bufs=2) as m1_p,
        tc.tile_pool(name="m2", bufs=2) as m2_p,
        tc.tile_pool(name="et", bufs=12) as et_p,
        tc.tile_pool(name="se", bufs=5) as se_p,
        tc.tile_pool(name="t2", bufs=2) as t2_p,
        tc.tile_pool(name="rb", bufs=2) as rb_p,
        tc.tile_pool(name="psQK", bufs=2, space="PSUM") as psQK,
        tc.tile_pool(name="psS", bufs=2, space="PSUM") as psS,
        tc.tile_pool(name="psY", bufs=1, space="PSUM") as psY,
    ):
        ones128 = const_p.tile([128, 128], bf16)
        nc.vector.memset(ones128[:], 1.0)
        csg = const_p.tile([128, T], bf16)
        ssg = const_p.tile([128, T], bf16)
        mask_sb = const_p.tile([128, 2, 256], bf16)

        v_sb = vsb_p.tile([128, NTT, NKV * HD], bf16)
        yts = [yts_p.tile([128, T], bf16, tag="yt", name=f"yt{i}")
               for i in range(NQ)]
        qkT: dict = {}

        # ---- attention emission helpers (interleaved into qkv slots) ----
        et_tiles: dict = {}
        se_tiles: dict = {}

        def scores_steps(g, r):
            """Emission-step closures: scoresT + exp + mask + pair-tree sums
            for q head (g, r). One step per si pair (10 total)."""
            hq = 4 * g + r
            ew = nc.vector

            def mk(c, pair):
                def step():
                    qt = qkT[hq]
                    kt = qkT[NQ + g]
                    np_ = c + 1
                    pss = psS.tile([128, 2, 256], f32, tag="psS")
                    et = et_p.tile([128, 2, 256], bf16, tag="et")
                    if pair == c:
                        # diagonal pair: si=2c covers all 256 cols, si=2c+1
                        # only the upper 128 (rest fully masked -> zeros)
                        nc.tensor.matmul(
                            pss[:, 0, :], kt[:, ts(2 * pair, 128)],
                            qt[:, ts(c, 256)], start=True, stop=True,
                        )
                        nc.tensor.matmul(
                            pss[:, 1, 128:256], kt[:, ts(2 * pair + 1, 128)],
                            qt[:, 256 * c + 128 : 256 * c + 256],
                            start=True, stop=True,
                        )
                        nc.vector.memset(et[:, 1, 0:128], 0.0)
                        nc.scalar.activation(
                            out=et[:, 0, :], in_=pss[:, 0, :], func=Exp,
                            scale=SCALE,
                        )
                        nc.scalar.activation(
                            out=et[:, 1, 128:256], in_=pss[:, 1, 128:256],
                            func=Exp, scale=SCALE,
                        )
                    else:
                        for i in range(2):
                            si = 2 * pair + i
                            nc.tensor.matmul(
                                pss[:, i, :],
                                kt[:, ts(si, 128)],
                                qt[:, ts(c, 256)],
                                start=True,
                                stop=True,
                            )
                        nc.scalar.activation(
                            out=et[:], in_=pss[:], func=Exp, scale=SCALE,
                        )
                    pairs = et_tiles.setdefault((hq, c), [])
                    pairs.append(et)
                    if pair == c:  # diagonal pair: mask, then finish the sum
                        ew.tensor_mul(et[:], et[:], mask_sb[:])
                        se = se_p.tile([128, 256], bf16, tag="se")
                        if np_ == 1:
                            ew.tensor_add(se[:], et[:, 0, :], et[:, 1, :])
                        else:
                            t2 = t2_p.tile([128, 2, 256], bf16, tag="t2")
                            ew.tensor_add(t2[:], pairs[0][:], pairs[1][:])
                            for k in range(2, np_):
                                ew.tensor_add(t2[:], t2[:], pairs[k][:])
                            ew.tensor_add(se[:], t2[:, 0, :], t2[:, 1, :])
                        se_tiles[(hq, c)] = se
                return step

            return [mk(c, pair) for c in range(4) for pair in range(c + 1)]

        def avdn_steps(g, r):
            """Emission-step closures: AV + broadcast denominator + normalize
            for q head (g, r). One step per chunk plus one per chunk-pair."""
            hq = 4 * g + r
            state: dict = {}

            def mk_av(c):
                def step():
                    c2, i = divmod(c, 2)
                    if i == 0:
                        state["py"] = psY.tile([128, 2, 256], f32, tag="psY", name="py")
                        state["dn"] = psS.tile([128, 2, 256], f32, tag="psS", name="dn")
                    py, dn = state["py"], state["dn"]
                    ns = 2 * (c + 1)
                    pairs = et_tiles.pop((hq, c))
                    for si in range(ns):
                        if si == ns - 1:
                            # odd-diagonal block: lower 128 tq cols are zero
                            nc.tensor.matmul(
                                py[:, i, 128:256],
                                v_sb[:, si, ts(g, 128)],
                                pairs[si // 2][:, si % 2, 128:256],
                                start=False,
                                stop=True,
                            )
                            continue
                        nc.tensor.matmul(
                            py[:, i, :],
                            v_sb[:, si, ts(g, 128)],
                            pairs[si // 2][:, si % 2, :],
                            start=(si == 0),
                            stop=False,
                        )
                    se = se_tiles.pop((hq, c))
                    nc.tensor.matmul(
                        dn[:, i, :], ones128[:], se[:], start=True, stop=True
                    )
                return step

            def mk_norm(c2):
                def step():
                    py, dn = state["py"], state["dn"]
                    rb = rb_p.tile([128, 2, 256], f32, tag="rb")
                    nc.vector.reciprocal(out=rb[:], in_=dn[:])
                    nc.vector.tensor_mul(
                        yts[hq][:, ts(c2, 512)].rearrange(
                            "p (a b) -> p a b", a=2),
                        py[:], rb[:],
                    )
                return step

            out = []
            for c2 in range(2):
                out.append(mk_av(2 * c2))
                out.append(mk_av(2 * c2 + 1))
                out.append(mk_norm(c2))
            return out

        def interleave(primary, inserts):
            """Emit primary closures with inserts spread evenly between."""
            n, m = len(primary), len(inserts)
            ii = 0
            for k, p in enumerate(primary):
                p()
                want = (k + 1) * m // n
                while ii < want:
                    inserts[ii]()
                    ii += 1
            while ii < m:
                inserts[ii]()
                ii += 1

        # ================= PHASE A+B: qkv + interleaved attention =========
        with (
            tc.tile_pool(name="x", bufs=1) as x_p,
            tc.tile_pool(name="wv", bufs=4) as wv_p,
            tc.tile_pool(name="wqk", bufs=3) as wqk_p,
        ):
            # head order per group: [k_g, q_{4g}, .., q_{4g+3}]
            def head_list(g):
                return [NQ + g, 4 * g, 4 * g + 1, 4 * g + 2, 4 * g + 3]

            wqk_t: dict = {}

            def fetch_w(h, half):
                wqk_t[(h, half)] = wqk_p.tile(
                    [128, 16, 128], bf16, tag="wqk", name=f"wqk{h}_{half}")
                nc.sync.dma_start(
                    out=wqk_t[(h, half)][:], in_=wqk_d[h, :, ts(half, 16), :])

            # DMA priority order, staggered so the k0/v-tt0 startup chains
            # are fed as early as possible: k0 W, then alternating x chunks
            # and v weights, then the small constants.
            fetch_w(head_list(0)[0], 0)
            xT_sb = x_p.tile([128, NCC, T], bf16)
            xr = xT_d.rearrange("(cc p) t -> p cc t", p=128)

            def fetch_x(ch):
                nc.sync.dma_start(
                    out=xT_sb[:, ts(ch, 4), :], in_=xr[:, ts(ch, 4), :]
                )

            wv_t = {}

            def fetch_wv(ch):
                wv_t[ch] = wv_p.tile([128, 8, NKV * HD], bf16, tag="wv",
                                     name=f"wv{ch}")
                nc.sync.dma_start(out=wv_t[ch][:], in_=wv_d[:, ts(ch, 8), :])

            fetch_x(0)
            fetch_wv(0)
            fetch_x(1)
            fetch_w(head_list(0)[0], 1)
            fetch_x(2)
            fetch_wv(1)
            fetch_x(3)
            fetch_x(4)
            fetch_wv(2)
            fetch_x(5)
            fetch_x(6)
            fetch_wv(3)
            fetch_x(7)
            nc.sync.dma_start(out=csg[:], in_=csg_d[:])
            nc.sync.dma_start(out=ssg[:], in_=ssg_d[:])
            nc.sync.dma_start(out=mask_sb[:], in_=mask_d[:])

            # v psum chains cycle through the two psS slots (idle during
            # group 0) plus the dedicated psV bank -> 3 concurrent chains
            v_pool_cycle = ["psS", "psS", "psV"]

            def v_steps(tt):
                """One v token-tile: 32-step accumulation + PSUM->SBUF copy."""
                vtag = v_pool_cycle[tt % 3]
                psv = (psS if vtag == "psS" else psQK).tile(
                    [128, 512], f32, tag=vtag, bufs=(1 if vtag == "psV" else None),
                    name=f"psv{tt}")

                def mk_vcc(cc):
                    def step():
                        nc.tensor.matmul(
                            psv[:],
                            xT_sb[:, cc, ts(tt, 128)],
                            wv_t[cc // 8][:, cc % 8, :],
                            start=(cc == 0),
                            stop=(cc == NCC - 1),
                        )
                        if cc == NCC - 1:
                            nc.scalar.copy(out=v_sb[:, tt, :], in_=psv[:])
                    return step

                return [mk_vcc(cc) for cc in range(NCC)]

            def emit_slot(g, qk_steps, ins):
                if g == 0:
                    # per-chunk lockstep: all chains consume x chunk m
                    # before any chain touches chunk m+1 (DMA pacing)
                    for ch in range(8):
                        for s in qk_steps[4 * ch : 4 * ch + 4]:
                            s()
                        for chain in ins:
                            for s in chain[4 * ch : 4 * ch + 4]:
                                s()
                else:
                    interleave(qk_steps, ins)

            # ---- q/k heads in groups; attention + v interleaved into slots
            def slot_inserts(g, j):
                ins = []
                if g == 0:
                    # v tiles: tt0-tt2 ride with k0 (fills the DMA-paced
                    # startup window), the rest spread over the q slots.
                    # Returned as chains for per-chunk lockstep emission.
                    vmap = {0: [0, 1, 2], 1: [3, 4], 2: [5], 3: [6], 4: [7]}
                    return [v_steps(tt) for tt in vmap[j]]
                # attention of the previous group: head j hosted in slot j
                # (the j=4 slot stays single-loaded for g3's early heads)
                if g == 3 and j >= 3:
                    ins += avdn_steps(3, j - 3)
                if j <= 3:
                    ins += scores_steps(g - 1, j)
                    ins += avdn_steps(g - 1, j)
                if g == 3 and j >= 2:
                    ins += scores_steps(3, j - 2)
                return ins

            for g in range(4):
                heads = head_list(g)
                for j, h in enumerate(heads):
                    # prefetch: this head's second half + next head's first
                    if (h, 1) not in wqk_t:
                        fetch_w(h, 1)
                    nxt = heads[j + 1] if j + 1 < 5 else (
                        head_list(g + 1)[0] if g + 1 < 4 else None)
                    if nxt is not None:
                        fetch_w(nxt, 0)
                    ps = psQK.tile([128, T], f32, tag="psQK")
                    wt0 = wqk_t.pop((h, 0))
                    wt1 = wqk_t.pop((h, 1))

                    def mk_cc(cc, wt, base):
                        def step():
                            nc.tensor.matmul(
                                ps[:, 0:512], wt[:, cc - base, :],
                                xT_sb[:, cc, 0:512],
                                start=(cc == 0), stop=(cc == NCC - 1),
                            )
                            nc.tensor.matmul(
                                ps[:, 512:1024], wt[:, cc - base, :],
                                xT_sb[:, cc, 512:1024],
                                start=(cc == 0), stop=(cc == NCC - 1),
                            )
                        return step

                    qk_steps = [mk_cc(cc, wt0, 0) for cc in range(16)]
                    qk_steps += [mk_cc(cc, wt1, 16) for cc in range(16, NCC)]
                    emit_slot(g, qk_steps, slot_inserts(g, j))
                    # RoPE: out = ps*csg + swap_halves(ps)*ssg
                    sw = sw_p.tile([128, T], bf16, tag="sw")
                    nc.scalar.copy(out=sw[0:64, :], in_=ps[64:128, :])
                    nc.scalar.copy(out=sw[64:128, :], in_=ps[0:64, :])
                    m1 = m1_p.tile([128, T], bf16, tag="m1")
                    nc.vector.tensor_mul(m1[:], ps[:], csg[:])
                    m2 = m2_p.tile([128, T], bf16, tag="m2")
                    nc.gpsimd.tensor_mul(m2[:], sw[:], ssg[:])
                    qt = qkT_p.tile([128, T], bf16, tag="qkT", name=f"qkT{h}")
                    nc.vector.tensor_add(qt[:], m1[:], m2[:])
                    qkT[h] = qt

        # ================= PHASE C: last-group attention + proj ===========
        with (
            tc.tile_pool(name="wpp", bufs=4) as wp_p,
            tc.tile_pool(name="ostage", bufs=3) as ostage_p,
        ):
            wp_t: dict = {}

            def fetch_wp(ccol):
                lo = wp_p.tile([128, 8, 512], bf16, tag="wp", name=f"wpl{ccol}")
                hi = wp_p.tile([128, 8, 512], bf16, tag="wp", name=f"wph{ccol}")
                nc.sync.dma_start(out=lo[:], in_=wp_d[ccol, :, 0:8, :])
                nc.sync.dma_start(out=hi[:], in_=wp_d[ccol, :, 8:16, :])
                wp_t[ccol] = (lo, hi)

            fetch_wp(0)

            def po_steps(ccol, tt, po, ycc_range):
                lo, hi = wp_t[ccol]

                def mk(ycc):
                    def step():
                        wtile = lo if ycc < 8 else hi
                        nc.tensor.matmul(
                            po[:],
                            yts[ycc][:, ts(tt, 128)],
                            wtile[:, ycc % 8, :],
                            start=(ycc == 0),
                            stop=(ycc == 15),
                        )
                        if ycc != 15:
                            return
                        ot = ostage_p.tile([128, 512], f32, tag="os")
                        nc.scalar.copy(out=ot[:], in_=po[:])
                        nc.sync.dma_start(
                            out=out_d[ts(tt, 128), ts(ccol, 512)],
                            in_=ot[:],
                        )
                    return step

                return [mk(ycc) for ycc in ycc_range]

            # remaining attention tail (the rest rode inside the B slots);
            # the first two proj accumulations' ycc 0-11 matmuls (which do
            # not depend on the last attention heads) fill the exp stalls.
            po0 = psQK.tile([128, 512], f32, tag="psQK", name="po0")
            po1 = psQK.tile([128, 512], f32, tag="psQK", name="po1")
            po2 = psQK.tile([128, 512], f32, tag="psV", bufs=1, name="po2")
            tail = (avdn_steps(3, 2) + scores_steps(3, 3) + avdn_steps(3, 3))
            interleave(tail, po_steps(0, 0, po0, range(12))
                       + po_steps(0, 1, po1, range(12))
                       + po_steps(0, 2, po2, range(12)))

            first = {0: (po0, 12), 1: (po1, 12), 2: (po2, 12)}
            for ccol in range(8):
                if ccol + 1 < 8:
                    fetch_wp(ccol + 1)
                for tt in range(NTT):
                    po, ystart = None, 0
                    if ccol == 0 and tt in first:
                        po, ystart = first[tt]
                    else:
                        po = psQK.tile([128, 512], f32, tag="psQK", name="po")
                    for s in po_steps(ccol, tt, po, range(ystart, 16)):
                        s()
                wp_t.pop(ccol)

    nc.compile()
    return nc


def prep_inputs(x, Wqkv, Wproj, freqs_cos, freqs_sin):
    """Build the 8 per-core input maps (host-side shard + layout prep)."""
    x = np.asarray(x, np.float32)
    Wqkv = np.asarray(Wqkv, np.float32)
    Wproj = np.asarray(Wproj, np.float32)
    cos = np.asarray(freqs_cos, np.float32)
    sin = np.asarray(freqs_sin, np.float32)

    perm = np.concatenate([np.arange(0, HD, 2), np.arange(1, HD, 2)])
    csg = np.ascontiguousarray(
        np.vstack([cos.T, cos.T]).astype(BF16))            # [128, T]
    ssg = np.ascontiguousarray(
        np.vstack([-sin.T, sin.T]).astype(BF16))           # [128, T]
    # mask[p, i, f] = 1.0 if 128*i + p <= f else 0 (diagonal 256-chunk pair)
    mask = (
        (128 * np.arange(2)[None, :, None] + np.arange(128)[:, None, None])
        <= np.arange(256)[None, None, :]
    ).astype(BF16)
    mask = np.ascontiguousarray(mask)

    in_maps = []
    for c in range(8):
        b, hh = divmod(c, 2)
        qcols = (hh * NQ * HD + (np.arange(NQ) * HD)[:, None] + perm[None, :]).ravel()
        kcols = (
            H * HD + hh * NKV * HD + (np.arange(NKV) * HD)[:, None] + perm[None, :]
        ).ravel()
        vcols = (
            (H + KV) * HD
            + hh * NKV * HD
            + (np.arange(NKV) * HD)[:, None]
            + np.arange(HD)[None, :]
        ).ravel()
        Wqk = Wqkv[:, np.concatenate([qcols, kcols])]      # [4096, 2560]
        # [h, p, cc, col]
        wqk = np.ascontiguousarray(
            Wqk.reshape(NCC, 128, QK_HEADS, 128).transpose(2, 1, 0, 3)
            .astype(BF16))
        Wv = Wqkv[:, vcols]                                # [4096, 512]
        wv = np.ascontiguousarray(
            Wv.reshape(NCC, 128, NKV * HD).transpose(1, 0, 2).astype(BF16))
        Wp = Wproj[hh * NQ * HD : (hh + 1) * NQ * HD, :]   # [2048, 4096]
        wp = np.ascontiguousarray(
            Wp.reshape(16, 128, 8, 512).transpose(2, 1, 0, 3).astype(BF16))
        xT = np.ascontiguousarray(x[b].T.astype(BF16))     # [4096, 1024]
        in_maps.append(
            {"xT": xT, "wqk": wqk, "wv": wv, "wp": wp,
             "csg": csg, "ssg": ssg, "maskd": mask}
        )
    return in_maps


def _get_nc():
    if "nc" not in _CACHE:
        _CACHE["nc"] = _build_nc()
    return _CACHE["nc"]


def kernel(x, Wqkv, Wproj, freqs_cos, freqs_sin, mask=None):
    from concourse.bass_utils import run_bass_kernel_spmd

    nc = _get_nc()
    in_maps = prep_inputs(x, Wqkv, Wproj, freqs_cos, freqs_sin)
    res = run_bass_kernel_spmd(nc, in_maps, core_ids=list(range(8)))
    outs = [res.results[c]["out"] for c in range(8)]
    y = np.stack([outs[2 * b] + outs[2 * b + 1] for b in range(B)], axis=0)
    return y.astype(np.float32)
